# revision 16
# baseline (speedup 1.0000x reference)
"""DySample (B=16,C=64,H=W=128, scale=2, groups=4) Trainium2 kernel — v5.

Fixed 4-tap stencil with data-dependent weights (derivation verified vs
reference). Per output quadrant (dy,dx) of group g, k = dx (g even) / dy:
  k=1: out = V + wx*HD + wy*VD + wx*wy*XD     (forward diffs)
  k=0: out = V + wx*HDm + wy*VDdn + wx*wy*XDL (backward diffs; signs folded
       into host-side conv weight/bias prep)

Measured-cost driven design (all bf16):
 - DVE bf16 TT runs 2x (~1.2us per [128,2048]) incl. flat-shifted reads and
   broadcast APs; quadrant pairs sharing k (same taps) are processed as
   single [128,4096] TTs (conv weight columns are permuted host-side so
   paired weight planes are adjacent).
 - xp is padded with clamp rows host-side so V, Vup, Vdn are three clean
   128-row HBM loads; all 6 diff tensors are DVE 2x TTs; clamped edge
   columns are exact zeros fixed by one tiny memset each.
 - Per quadrant-pair: DVE does 3 weight mults + adds; TensorEngine merges
   the remaining planes per quadrant in PSUM via identity matmuls (the
   engines are balanced: half the pairs add V on DVE, half on PE);
   ScalarEngine drains PSUM straight into the interleaved output buffer.
 - GpSimd does only tiny memsets (its big TTs are slow and poison DVE via
   shared SBUF ports). Loads on sync queue; output stores issued by scalar
   right after its own drains.
"""
import sys, types, ctypes, contextlib

sys.path.insert(0, "/opt/trn_rl_repo")

import numpy as np

_SO_PATH = "/opt/axon/libaxon_pjrt.so"


def _install_hooks():
    if "antenv.axon_hooks" in sys.modules:
        return
    mod = types.ModuleType("antenv.axon_hooks")
    mod._hook = None
    mod.set_axon_ntff_profile_hook = lambda h: setattr(mod, "_hook", h)
    mod.get_axon_ntff_profile_hook = lambda: mod._hook
    sys.modules["antenv.axon_hooks"] = mod
    try:
        lib = ctypes.CDLL(_SO_PATH)
        if not hasattr(lib, "axon_start_nrt_profile"):
            return
        lib.axon_start_nrt_profile.argtypes = [ctypes.POINTER(ctypes.c_int64), ctypes.c_size_t]
        lib.axon_start_nrt_profile.restype = ctypes.c_int64
        lib.axon_stop_nrt_profile.argtypes = [ctypes.c_char_p]
        lib.axon_stop_nrt_profile.restype = ctypes.c_int64

        @contextlib.contextmanager
        def _hook(output_dir, device_ids):
            import jax
            jax.devices()
            if device_ids:
                ids = (ctypes.c_int64 * len(device_ids))(*device_ids)
                rc = lib.axon_start_nrt_profile(ids, len(device_ids))
            else:
                rc = lib.axon_start_nrt_profile(None, 0)
            if rc != 0:
                raise RuntimeError(f"axon_start_nrt_profile rc={rc}")
            try:
                yield
            finally:
                lib.axon_stop_nrt_profile(str(output_dir).encode())

        mod.set_axon_ntff_profile_hook(_hook)
    except OSError:
        pass


_install_hooks()

import concourse.bass as bass
import concourse.bacc as bacc
import concourse.tile as tile
import concourse.mybir as mybir
from contextlib import ExitStack
from concourse.bass_utils import run_bass_kernel_spmd

f32 = mybir.dt.float32
bf16 = mybir.dt.bfloat16
Op = mybir.AluOpType

N_CORES = 8
B, C, H, W = 16, 64, 128, 128
BPC = B // N_CORES
G, S = 4, 2
NO = 32
CB = 16
FD = CB * W          # 2048
PADF = 16
FT = FD + 2 * PADF   # 2080

# plane permutation: paired quadrants' weight planes made adjacent.
# g even pairs over dy (o, o+2); g odd pairs over dx (o, o+1).
PERM = [0, 2, 1, 3, 4, 5, 6, 7, 8, 10, 9, 11, 12, 13, 14, 15]
POS = [0] * 16
for _i, _o in enumerate(PERM):
    POS[_o] = _i

_cache = {}


def _build():
    nc = bacc.Bacc("TRN2", target_bir_lowering=False, debug=False, num_devices=1)
    xb_ap = nc.dram_tensor("xb", [BPC * C, H * W], bf16, kind="ExternalInput").ap()
    xp_ap = nc.dram_tensor("xp", [BPC * G, H + 2, FD], bf16, kind="ExternalInput").ap()
    wg_ap = nc.dram_tensor("wg", [128, C], bf16, kind="ExternalInput").ap()
    sm_ap = nc.dram_tensor("sm", [128, 129], bf16, kind="ExternalInput").ap()
    out_ap = nc.dram_tensor("out", [BPC, C, 2 * H, 2 * W], bf16, kind="ExternalOutput").ap()
    scr_ap = nc.dram_tensor("scr", [C, H * W], bf16, kind="Internal").ap()

    with tile.TileContext(nc) as tc, ExitStack() as ctx:
        pool = ctx.enter_context(tc.tile_pool(name="p", bufs=1))
        pool2 = ctx.enter_context(tc.tile_pool(name="p2", bufs=2))
        pool3 = ctx.enter_context(tc.tile_pool(name="p3", bufs=2))

        smat = pool.tile([128, 129], bf16, tag="smat")
        nc.sync.dma_start(smat[:], sm_ap[:])
        SI = smat[:, 0:128]
        bias = smat[0:C, 128:129]

        waug = pool.tile([128, C], bf16, tag="waug")
        nc.sync.dma_start(waug[:], wg_ap[:])

        off_y = pool.tile([128, C * W], bf16, tag="offy")
        wxy = pool.tile([128, BPC * CB * W], bf16, tag="wxy")

        # ---------- conv (folded+permuted weight planes, o-major) ----------
        with tc.tile_pool(name="pc", bufs=2, space="PSUM") as ppc:
            for h4 in range(8):
                xc = pool2.tile([128, 2048], bf16, tag="xc")
                nc.sync.dma_start(xc[:], xb_ap[:, bass.ts(h4, 2048)])
                for q in range(4):
                    ps = ppc.tile([C, 512], f32, tag="cps")
                    nc.tensor.matmul(ps[:], waug[:], xc[:, bass.ts(q, 512)],
                                     start=True, stop=True)
                    ck = pool2.tile([C, 512], bf16, tag="ck")
                    nc.scalar.activation(
                        ck[:], ps[:],
                        mybir.ActivationFunctionType.Identity, bias=bias)
                    nc.gpsimd.dma_start(scr_ap[:, bass.ts(h4 * 4 + q, 512)], ck[:])

        # o-major -> y-major (DRAM bounce read-back)
        nc.sync.dma_start(
            off_y[:].rearrange("y (i x) -> y i x", i=C),
            scr_ap[:].rearrange("i (y x) -> y i x", x=W))
        for b in range(BPC):
            nc.vector.tensor_tensor(
                wxy[:, bass.ts(b, 2048)],
                off_y[:, b * 4096: b * 4096 + 2048],
                off_y[:, b * 4096 + 2048: b * 4096 + 4096], Op.mult)

        # ---------- stencil ----------
        with tc.tile_pool(name="pp", bufs=2, space="PSUM") as pps:
            for b in range(BPC):
                for g in range(G):
                    Vc = pool2.tile([128, FT], bf16, tag="Vc")
                    V0 = Vc[:, PADF:PADF + FD]
                    nc.sync.dma_start(V0, xp_ap[b * G + g, 1:129])
                    Vup = pool2.tile([128, FD], bf16, tag="Vup")
                    Vdn = pool2.tile([128, FD], bf16, tag="Vdn")
                    nc.sync.dma_start(Vup[:], xp_ap[b * G + g, 2:130])
                    nc.sync.dma_start(Vdn[:], xp_ap[b * G + g, 0:128])

                    VD = pool3.tile([128, FT], bf16, tag="VD")
                    VDdn = pool3.tile([128, FT], bf16, tag="VDdn")
                    HD = pool3.tile([128, FD], bf16, tag="HD")
                    HDm = pool3.tile([128, FD], bf16, tag="HDm")
                    XD = pool3.tile([128, FD], bf16, tag="XD")
                    XDL = pool3.tile([128, FD], bf16, tag="XDL")
                    VD0 = VD[:, PADF:PADF + FD]
                    VDdn0 = VDdn[:, PADF:PADF + FD]
                    nc.vector.tensor_tensor(VD0, Vup[:], V0, Op.subtract)
                    nc.vector.tensor_tensor(VDdn0, V0, Vdn[:], Op.subtract)
                    nc.vector.tensor_tensor(HD[:], Vc[:, PADF + 1:PADF + 1 + FD], V0, Op.subtract)
                    nc.vector.tensor_tensor(HDm[:], V0, Vc[:, PADF - 1:PADF - 1 + FD], Op.subtract)
                    nc.vector.tensor_tensor(XD[:], VD[:, PADF + 1:PADF + 1 + FD], VD0, Op.subtract)
                    nc.vector.tensor_tensor(XDL[:], VDdn0, VDdn[:, PADF - 1:PADF - 1 + FD], Op.subtract)
                    for t in (HD, XD):
                        nc.gpsimd.memset(
                            t[:].rearrange("y (c x) -> y c x", x=W)[:, :, 127:128], 0.0)
                    for t in (HDm, XDL):
                        nc.gpsimd.memset(
                            t[:].rearrange("y (c x) -> y c x", x=W)[:, :, 0:1], 0.0)

                    def b4(ap3):  # [y,c,x] tap -> broadcast over pair dim
                        return ap3.unsqueeze(1).broadcast_to([128, 2, CB, W])

                    tHD = b4(HD[:].rearrange("y (c x) -> y c x", x=W))
                    tHDm = b4(HDm[:].rearrange("y (c x) -> y c x", x=W))
                    tVD = b4(VD0.rearrange("y (c x) -> y c x", x=W))
                    tVDdn = b4(VDdn0.rearrange("y (c x) -> y c x", x=W))
                    tXD = b4(XD[:].rearrange("y (c x) -> y c x", x=W))
                    tXDL = b4(XDL[:].rearrange("y (c x) -> y c x", x=W))
                    Vb = V0.unsqueeze(1).broadcast_to([128, 2, FD])

                    AS2 = pool2.tile([128, CB * 2 * 2 * W], bf16, tag="AS2")
                    ASv = AS2[:].rearrange("y (c d x two) -> y c d x two",
                                           c=CB, d=2, two=2)

                    for pr in range(2):
                        if g % 2 == 0:
                            quads = [(0, pr), (1, pr)]
                        else:
                            quads = [(pr, 0), (pr, 1)]
                        k = pr
                        oA = g * 4 + quads[0][0] * 2 + quads[0][1]
                        oB = g * 4 + quads[1][0] * 2 + quads[1][1]
                        piA = POS[oA]
                        assert POS[oB] == piA + 1
                        colA = (b * 32 + piA) * W

                        def wb2(src, c0):
                            return src[:, c0:c0 + 2 * W].rearrange(
                                "y (q x) -> y q x", x=W).unsqueeze(2).broadcast_to([128, 2, CB, W])

                        wxb = wb2(off_y, colA)
                        wyb = wb2(off_y, colA + 16 * W)
                        wxyb = wb2(wxy, (b * 16 + piA) * W)
                        if k == 1:
                            tx, ty, tcr = tHD, tVD, tXD
                        else:
                            tx, ty, tcr = tHDm, tVDdn, tXDL

                        m1 = pool3.tile([128, 2 * FD], bf16, tag="m1")
                        m2 = pool3.tile([128, 2 * FD], bf16, tag="m2")
                        mc = pool3.tile([128, 2 * FD], bf16, tag="mc")
                        m1v = m1[:].rearrange("y (q c x) -> y q c x", q=2, x=W)
                        m2v = m2[:].rearrange("y (q c x) -> y q c x", q=2, x=W)
                        mcv = mc[:].rearrange("y (q c x) -> y q c x", q=2, x=W)
                        # mc first: PE's opening round only depends on it
                        nc.vector.tensor_tensor(mcv, tcr, wxyb, Op.mult)
                        nc.vector.tensor_tensor(m1v, tx, wxb, Op.mult)
                        nc.vector.tensor_tensor(m2v, ty, wyb, Op.mult)
                        nc.vector.tensor_tensor(m1[:], m1[:], m2[:], Op.add)
                        three_round = not (pr == 0 and g % 2 == 0)
                        if not three_round:
                            # V folded on DVE -> only 2 PE rounds per quadrant
                            nc.vector.tensor_tensor(
                                m1[:].rearrange("y (q f) -> y q f", q=2),
                                m1[:].rearrange("y (q f) -> y q f", q=2),
                                Vb, Op.add)

                        for qi, (dy, dx) in enumerate(quads):
                            qp = pps.tile([128, 2048], f32, tag="ps")
                            for cc in range(4):
                                nc.tensor.matmul(qp[:, bass.ts(cc, 512)], SI,
                                                 mc[:, qi * FD + 512 * cc: qi * FD + 512 * (cc + 1)],
                                                 start=True, stop=False)
                            if three_round:
                                for cc in range(4):
                                    nc.tensor.matmul(qp[:, bass.ts(cc, 512)], SI,
                                                     V0[:, bass.ts(cc, 512)],
                                                     start=False, stop=False)
                            for cc in range(4):
                                nc.tensor.matmul(qp[:, bass.ts(cc, 512)], SI,
                                                 m1[:, qi * FD + 512 * cc: qi * FD + 512 * (cc + 1)],
                                                 start=False, stop=True)
                            nc.scalar.copy(
                                ASv[:, :, dy, :, dx],
                                qp[:].rearrange("y (c x) -> y c x", x=W))

                    nc.scalar.dma_start(
                        out_ap[b, g * CB:(g + 1) * CB].rearrange(
                            "c (y d) x -> y c d x", d=2),
                        AS2[:].rearrange("y (c d x) -> y c d x", c=CB, d=2))

    nc.compile()
    return nc


def _host_prep(x, w_off, b_off):
    import ml_dtypes
    nbf = ml_dtypes.bfloat16
    x = np.asarray(x, dtype=np.float32)

    w = 0.25 * np.asarray(w_off, dtype=np.float32)
    bb = 0.25 * np.asarray(b_off, dtype=np.float32)
    bf = bb.copy()
    for o in range(16):
        g, r = divmod(o, 4)
        dy, dx = divmod(r, 2)
        k = dx if g % 2 == 0 else dy
        sgn = 1.0 if k == 1 else -1.0
        bf[o] = bb[o] + sgn * 0.25
        bf[16 + o] = bb[16 + o] + sgn * 0.25
    # permute planes so paired quadrants' planes are adjacent
    wp = np.empty_like(w)
    bp = np.empty_like(bf)
    for i, o in enumerate(PERM):
        wp[i] = w[o]
        wp[16 + i] = w[16 + o]
        bp[i] = bf[o]
        bp[16 + i] = bf[16 + o]
    waug = np.zeros((128, 64), dtype=np.float32)
    waug[0:64, 0:32] = wp.T
    waug[64:128, 32:64] = wp.T
    wg = waug.astype(nbf)

    sm = np.zeros((128, 129), dtype=np.float32)
    sm[:, 0:128] = np.eye(128, dtype=np.float32)
    sm[0:64, 128] = np.concatenate([bp, bp])
    sm = sm.astype(nbf)

    xbf = x.astype(nbf)
    xg = xbf.reshape(B, G, CB, H, W).transpose(0, 1, 3, 2, 4)
    xpre = np.empty((B, G, H + 2, CB, W), dtype=nbf)
    xpre[:, :, 1:H + 1] = xg
    xpre[:, :, 0] = xg[:, :, 0]
    xpre[:, :, H + 1] = xg[:, :, H - 1]
    xpre = np.ascontiguousarray(xpre.reshape(B, G, H + 2, CB * W))
    xbc = np.ascontiguousarray(xbf.reshape(B, C, H * W))
    return xbc, xpre, wg, sm


def kernel(x, w_off, b_off):
    key = "k"
    if key not in _cache:
        _cache[key] = _build()
    nc = _cache[key]

    xbc, xpre, wg, sm = _host_prep(x, w_off, b_off)
    in_maps = []
    for i in range(N_CORES):
        xb = xbc[BPC * i:BPC * (i + 1)].reshape(BPC * C, H * W)
        xp = xpre[BPC * i:BPC * (i + 1)].reshape(BPC * G, H + 2, CB * W)
        in_maps.append({"xb": np.ascontiguousarray(xb),
                        "xp": np.ascontiguousarray(xp),
                        "wg": wg, "sm": sm})

    res = run_bass_kernel_spmd(nc, in_maps, core_ids=list(range(N_CORES)))
    out = np.empty((B, C, 2 * H, 2 * W), dtype=np.float32)
    for i in range(N_CORES):
        out[BPC * i:BPC * (i + 1)] = np.asarray(
            res.results[i]["out"], dtype=np.float32)
    return out


# revision 22
# speedup vs baseline: 1.0607x; 1.0607x over previous
"""DySample (B=16,C=64,H=W=128, scale=2, groups=4) Trainium2 kernel — v5.

Fixed 4-tap stencil with data-dependent weights (derivation verified vs
reference). Per output quadrant (dy,dx) of group g, k = dx (g even) / dy:
  k=1: out = V + wx*HD + wy*VD + wx*wy*XD     (forward diffs)
  k=0: out = V + wx*HDm + wy*VDdn + wx*wy*XDL (backward diffs; signs folded
       into host-side conv weight/bias prep)

Measured-cost driven design (all bf16):
 - DVE bf16 TT runs 2x (~1.2us per [128,2048]) incl. flat-shifted reads and
   broadcast APs; quadrant pairs sharing k (same taps) are processed as
   single [128,4096] TTs (conv weight columns are permuted host-side so
   paired weight planes are adjacent).
 - xp is padded with clamp rows host-side so V, Vup, Vdn are three clean
   128-row HBM loads; all 6 diff tensors are DVE 2x TTs; clamped edge
   columns are exact zeros fixed by one tiny memset each.
 - Per quadrant-pair: DVE does 3 weight mults + adds; TensorEngine merges
   the remaining planes per quadrant in PSUM via identity matmuls (the
   engines are balanced: half the pairs add V on DVE, half on PE);
   ScalarEngine drains PSUM straight into the interleaved output buffer.
 - GpSimd does only tiny memsets (its big TTs are slow and poison DVE via
   shared SBUF ports). Loads on sync queue; output stores issued by scalar
   right after its own drains.
"""
import sys, types, ctypes, contextlib

sys.path.insert(0, "/opt/trn_rl_repo")

import numpy as np

_SO_PATH = "/opt/axon/libaxon_pjrt.so"


def _install_hooks():
    if "antenv.axon_hooks" in sys.modules:
        return
    mod = types.ModuleType("antenv.axon_hooks")
    mod._hook = None
    mod.set_axon_ntff_profile_hook = lambda h: setattr(mod, "_hook", h)
    mod.get_axon_ntff_profile_hook = lambda: mod._hook
    sys.modules["antenv.axon_hooks"] = mod
    try:
        lib = ctypes.CDLL(_SO_PATH)
        if not hasattr(lib, "axon_start_nrt_profile"):
            return
        lib.axon_start_nrt_profile.argtypes = [ctypes.POINTER(ctypes.c_int64), ctypes.c_size_t]
        lib.axon_start_nrt_profile.restype = ctypes.c_int64
        lib.axon_stop_nrt_profile.argtypes = [ctypes.c_char_p]
        lib.axon_stop_nrt_profile.restype = ctypes.c_int64

        @contextlib.contextmanager
        def _hook(output_dir, device_ids):
            import jax
            jax.devices()
            if device_ids:
                ids = (ctypes.c_int64 * len(device_ids))(*device_ids)
                rc = lib.axon_start_nrt_profile(ids, len(device_ids))
            else:
                rc = lib.axon_start_nrt_profile(None, 0)
            if rc != 0:
                raise RuntimeError(f"axon_start_nrt_profile rc={rc}")
            try:
                yield
            finally:
                lib.axon_stop_nrt_profile(str(output_dir).encode())

        mod.set_axon_ntff_profile_hook(_hook)
    except OSError:
        pass


_install_hooks()

import concourse.bass as bass
import concourse.bacc as bacc
import concourse.tile as tile
import concourse.mybir as mybir
from contextlib import ExitStack
from concourse.bass_utils import run_bass_kernel_spmd

f32 = mybir.dt.float32
bf16 = mybir.dt.bfloat16
Op = mybir.AluOpType

N_CORES = 8
B, C, H, W = 16, 64, 128, 128
BPC = B // N_CORES
G, S = 4, 2
NO = 32
CB = 16
FD = CB * W          # 2048
PADF = 16
FT = FD + 2 * PADF   # 2080

# plane permutation: paired quadrants' weight planes made adjacent.
# g even pairs over dy (o, o+2); g odd pairs over dx (o, o+1).
PERM = [0, 2, 1, 3, 4, 5, 6, 7, 8, 10, 9, 11, 12, 13, 14, 15]
POS = [0] * 16
for _i, _o in enumerate(PERM):
    POS[_o] = _i

_cache = {}


def _build():
    nc = bacc.Bacc("TRN2", target_bir_lowering=False, debug=False, num_devices=1)
    xb_ap = nc.dram_tensor("xb", [BPC * C, H * W], bf16, kind="ExternalInput").ap()
    xp_ap = nc.dram_tensor("xp", [BPC * G, H + 2, FD], bf16, kind="ExternalInput").ap()
    wg_ap = nc.dram_tensor("wg", [128, C], bf16, kind="ExternalInput").ap()
    sm_ap = nc.dram_tensor("sm", [128, 129], bf16, kind="ExternalInput").ap()
    out_ap = nc.dram_tensor("out", [BPC, C, 2 * H, 2 * W], bf16, kind="ExternalOutput").ap()
    scr_ap = nc.dram_tensor("scr", [C, H * W], bf16, kind="Internal").ap()

    with tile.TileContext(nc) as tc, ExitStack() as ctx:
        pool = ctx.enter_context(tc.tile_pool(name="p", bufs=1))
        pool2 = ctx.enter_context(tc.tile_pool(name="p2", bufs=2))
        pool3 = ctx.enter_context(tc.tile_pool(name="p3", bufs=2))
        pool4 = ctx.enter_context(tc.tile_pool(name="p4", bufs=4))

        smat = pool.tile([128, 129], bf16, tag="smat")
        nc.sync.dma_start(smat[:], sm_ap[:])
        SI = smat[:, 0:128]
        bias = smat[0:C, 128:129]

        waug = pool.tile([128, C], bf16, tag="waug")
        nc.sync.dma_start(waug[:], wg_ap[:])

        off_y = pool.tile([128, C * W], bf16, tag="offy")
        wxy = pool.tile([128, BPC * CB * W], bf16, tag="wxy")

        # ---------- conv (folded+permuted weight planes, o-major) ----------
        with tc.tile_pool(name="pc", bufs=4, space="PSUM") as ppc:
            for h4 in range(8):
                xc = pool2.tile([128, 2048], bf16, tag="xc")
                nc.sync.dma_start(xc[:], xb_ap[:, bass.ts(h4, 2048)])
                for q in range(4):
                    ps = ppc.tile([C, 512], f32, tag="cps")
                    nc.tensor.matmul(ps[:], waug[:], xc[:, bass.ts(q, 512)],
                                     start=True, stop=True)
                    ck = pool4.tile([C, 512], bf16, tag="ck")
                    nc.scalar.activation(
                        ck[:], ps[:],
                        mybir.ActivationFunctionType.Identity, bias=bias)
                    nc.gpsimd.dma_start(scr_ap[:, bass.ts(h4 * 4 + q, 512)], ck[:])

        # o-major -> y-major (DRAM bounce read-back)
        nc.gpsimd.dma_start(
            off_y[:].rearrange("y (i x) -> y i x", i=C),
            scr_ap[:].rearrange("i (y x) -> y i x", x=W))
        wxy_done = [False] * BPC

        def ensure_wxy(b):
            if not wxy_done[b]:
                nc.vector.tensor_tensor(
                    wxy[:, bass.ts(b, 2048)],
                    off_y[:, b * 4096: b * 4096 + 2048],
                    off_y[:, b * 4096 + 2048: b * 4096 + 4096], Op.mult)
                wxy_done[b] = True

        # ---------- stencil (software-pipelined: taps one iter ahead) ----------
        def taps_block(b, g):
            Vc = pool2.tile([128, FT], bf16, tag="Vc")
            V0 = Vc[:, PADF:PADF + FD]
            nc.sync.dma_start(V0, xp_ap[b * G + g, 1:129])
            Vup = pool2.tile([128, FD], bf16, tag="Vup")
            Vdn = pool2.tile([128, FD], bf16, tag="Vdn")
            nc.sync.dma_start(Vup[:], xp_ap[b * G + g, 2:130])
            nc.sync.dma_start(Vdn[:], xp_ap[b * G + g, 0:128])

            VD = pool3.tile([128, FT], bf16, tag="VD")
            VDdn = pool3.tile([128, FT], bf16, tag="VDdn")
            HD = pool3.tile([128, FD], bf16, tag="HD")
            HDm = pool3.tile([128, FD], bf16, tag="HDm")
            XD = pool3.tile([128, FD], bf16, tag="XD")
            XDL = pool3.tile([128, FD], bf16, tag="XDL")
            VD0 = VD[:, PADF:PADF + FD]
            VDdn0 = VDdn[:, PADF:PADF + FD]
            nc.vector.tensor_tensor(VD0, Vup[:], V0, Op.subtract)
            nc.vector.tensor_tensor(VDdn0, V0, Vdn[:], Op.subtract)
            nc.vector.tensor_tensor(HD[:], Vc[:, PADF + 1:PADF + 1 + FD], V0, Op.subtract)
            nc.vector.tensor_tensor(HDm[:], V0, Vc[:, PADF - 1:PADF - 1 + FD], Op.subtract)
            nc.vector.tensor_tensor(XD[:], VD[:, PADF + 1:PADF + 1 + FD], VD0, Op.subtract)
            nc.vector.tensor_tensor(XDL[:], VDdn0, VDdn[:, PADF - 1:PADF - 1 + FD], Op.subtract)
            for t in (HD, XD):
                nc.gpsimd.memset(
                    t[:].rearrange("y (c x) -> y c x", x=W)[:, :, 127:128], 0.0)
            for t in (HDm, XDL):
                nc.gpsimd.memset(
                    t[:].rearrange("y (c x) -> y c x", x=W)[:, :, 0:1], 0.0)

            def b4(ap3):  # [y,c,x] tap -> broadcast over pair dim
                return ap3.unsqueeze(1).broadcast_to([128, 2, CB, W])

            taps = {
                1: (b4(HD[:].rearrange("y (c x) -> y c x", x=W)),
                    b4(VD0.rearrange("y (c x) -> y c x", x=W)),
                    b4(XD[:].rearrange("y (c x) -> y c x", x=W))),
                0: (b4(HDm[:].rearrange("y (c x) -> y c x", x=W)),
                    b4(VDdn0.rearrange("y (c x) -> y c x", x=W)),
                    b4(XDL[:].rearrange("y (c x) -> y c x", x=W))),
            }
            return (b, g, V0, V0.unsqueeze(1).broadcast_to([128, 2, FD]), taps)

        def quads_block(st, pps):
            b, g, V0, Vb, tapsets = st
            ensure_wxy(b)
            AS2 = pool2.tile([128, CB * 2 * 2 * W], bf16, tag="AS2")
            ASv = AS2[:].rearrange("y (c d x two) -> y c d x two",
                                   c=CB, d=2, two=2)
            for pr in range(2):
                        if g % 2 == 0:
                            quads = [(0, pr), (1, pr)]
                        else:
                            quads = [(pr, 0), (pr, 1)]
                        k = pr
                        oA = g * 4 + quads[0][0] * 2 + quads[0][1]
                        oB = g * 4 + quads[1][0] * 2 + quads[1][1]
                        piA = POS[oA]
                        assert POS[oB] == piA + 1
                        colA = (b * 32 + piA) * W

                        def wb2(src, c0):
                            return src[:, c0:c0 + 2 * W].rearrange(
                                "y (q x) -> y q x", x=W).unsqueeze(2).broadcast_to([128, 2, CB, W])

                        wxb = wb2(off_y, colA)
                        wyb = wb2(off_y, colA + 16 * W)
                        wxyb = wb2(wxy, (b * 16 + piA) * W)
                        tx, ty, tcr = tapsets[k]

                        m1 = pool3.tile([128, 2 * FD], bf16, tag="m1")
                        m2 = pool3.tile([128, 2 * FD], bf16, tag="m2")
                        mc = pool3.tile([128, 2 * FD], bf16, tag="mc")
                        m1v = m1[:].rearrange("y (q c x) -> y q c x", q=2, x=W)
                        m2v = m2[:].rearrange("y (q c x) -> y q c x", q=2, x=W)
                        mcv = mc[:].rearrange("y (q c x) -> y q c x", q=2, x=W)
                        # mc first: PE's opening round only depends on it
                        nc.vector.tensor_tensor(mcv, tcr, wxyb, Op.mult)
                        nc.vector.tensor_tensor(m1v, tx, wxb, Op.mult)
                        nc.vector.tensor_tensor(m2v, ty, wyb, Op.mult)
                        nc.vector.tensor_tensor(m1[:], m1[:], m2[:], Op.add)
                        three_round = not (pr == 0 and g % 2 == 0)
                        if not three_round:
                            # V folded on DVE -> only 2 PE rounds per quadrant
                            nc.vector.tensor_tensor(
                                m1[:].rearrange("y (q f) -> y q f", q=2),
                                m1[:].rearrange("y (q f) -> y q f", q=2),
                                Vb, Op.add)

                        for qi, (dy, dx) in enumerate(quads):
                            qp = pps.tile([128, 2048], f32, tag="ps")
                            for cc in range(4):
                                nc.tensor.matmul(qp[:, bass.ts(cc, 512)], SI,
                                                 mc[:, qi * FD + 512 * cc: qi * FD + 512 * (cc + 1)],
                                                 start=True, stop=False)
                            if three_round:
                                for cc in range(4):
                                    nc.tensor.matmul(qp[:, bass.ts(cc, 512)], SI,
                                                     V0[:, bass.ts(cc, 512)],
                                                     start=False, stop=False)
                            for cc in range(4):
                                nc.tensor.matmul(qp[:, bass.ts(cc, 512)], SI,
                                                 m1[:, qi * FD + 512 * cc: qi * FD + 512 * (cc + 1)],
                                                 start=False, stop=True)
                            nc.scalar.copy(
                                ASv[:, :, dy, :, dx],
                                qp[:].rearrange("y (c x) -> y c x", x=W))

                    nc.scalar.dma_start(
                        out_ap[b, g * CB:(g + 1) * CB].rearrange(
                            "c (y d) x -> y c d x", d=2),
                        AS2[:].rearrange("y (c d x) -> y c d x", c=CB, d=2))

    nc.compile()
    return nc


def _host_prep(x, w_off, b_off):
    import ml_dtypes
    nbf = ml_dtypes.bfloat16
    x = np.asarray(x, dtype=np.float32)

    w = 0.25 * np.asarray(w_off, dtype=np.float32)
    bb = 0.25 * np.asarray(b_off, dtype=np.float32)
    bf = bb.copy()
    for o in range(16):
        g, r = divmod(o, 4)
        dy, dx = divmod(r, 2)
        k = dx if g % 2 == 0 else dy
        sgn = 1.0 if k == 1 else -1.0
        bf[o] = bb[o] + sgn * 0.25
        bf[16 + o] = bb[16 + o] + sgn * 0.25
    # permute planes so paired quadrants' planes are adjacent
    wp = np.empty_like(w)
    bp = np.empty_like(bf)
    for i, o in enumerate(PERM):
        wp[i] = w[o]
        wp[16 + i] = w[16 + o]
        bp[i] = bf[o]
        bp[16 + i] = bf[16 + o]
    waug = np.zeros((128, 64), dtype=np.float32)
    waug[0:64, 0:32] = wp.T
    waug[64:128, 32:64] = wp.T
    wg = waug.astype(nbf)

    sm = np.zeros((128, 129), dtype=np.float32)
    sm[:, 0:128] = np.eye(128, dtype=np.float32)
    sm[0:64, 128] = np.concatenate([bp, bp])
    sm = sm.astype(nbf)

    xbf = x.astype(nbf)
    xg = xbf.reshape(B, G, CB, H, W).transpose(0, 1, 3, 2, 4)
    xpre = np.empty((B, G, H + 2, CB, W), dtype=nbf)
    xpre[:, :, 1:H + 1] = xg
    xpre[:, :, 0] = xg[:, :, 0]
    xpre[:, :, H + 1] = xg[:, :, H - 1]
    xpre = np.ascontiguousarray(xpre.reshape(B, G, H + 2, CB * W))
    xbc = np.ascontiguousarray(xbf.reshape(B, C, H * W))
    return xbc, xpre, wg, sm


def kernel(x, w_off, b_off):
    key = "k"
    if key not in _cache:
        _cache[key] = _build()
    nc = _cache[key]

    xbc, xpre, wg, sm = _host_prep(x, w_off, b_off)
    in_maps = []
    for i in range(N_CORES):
        xb = xbc[BPC * i:BPC * (i + 1)].reshape(BPC * C, H * W)
        xp = xpre[BPC * i:BPC * (i + 1)].reshape(BPC * G, H + 2, CB * W)
        in_maps.append({"xb": np.ascontiguousarray(xb),
                        "xp": np.ascontiguousarray(xp),
                        "wg": wg, "sm": sm})

    res = run_bass_kernel_spmd(nc, in_maps, core_ids=list(range(N_CORES)))
    out = np.empty((B, C, 2 * H, 2 * W), dtype=np.float32)
    for i in range(N_CORES):
        out[BPC * i:BPC * (i + 1)] = np.asarray(
            res.results[i]["out"], dtype=np.float32)
    return out
